# revision 13
# baseline (speedup 1.0000x reference)
"""Trainium2 Bass kernel for the BrushStroke renderer.

out[b,c,h,w] = (1/N) * sum_n sum_{p,q} Fy[b,n,h,p] * patches[b,n,c,p,q] * Fx[b,n,w,q]

with Fx/Fy normalized Gaussian filter banks (sigma=0.2) over a padded 272-wide
coordinate grid.

Strategy (8 NeuronCores, data-parallel over batch B=32 -> 4 batches/core).
Two device graphs are built; kernel() picks per input:

"win" (primary, used when the static window check passes — true for i.i.d.
uniform stroke positions):
  - host sorts strokes per batch by gx (a pure permutation; the stroke mean
    is permutation-invariant) and packs transposed patch blocks into a
    block-diagonal [128, 8*128] fp16 operand per (b, c).
  - chunks of 8 strokes; SBUF partition dim = (stroke-in-chunk, patch-dim).
  - each chunk gets a STATIC 128-wide w-window (chunk j of the gx-sorted
    order lives near w = 32j); x-filters are computed only on the window.
  - x-filters: all-ScalarE chain Square(coords + (-mu)) -> PSUM, Exp ->
    fp16 SBUF with fused accum_out row sums (normalization for free).
  - y-filters: full 272-grid, DVE subtract -> GPSIMD square -> ScalarE Exp.
  - stage 1 (contract q): U'[(i,p), wl] = BigP2_k^T @ FxN_k, K=128-dense
    matmuls, all 8 chunks into one 2-bank PSUM tile; one fp32->fp16 copy.
  - stage 2 (contract i,p): out[h, w-window] += FyN_k^T @ U'_k, scattered
    into a pre-zeroed PSUM bank (K=1 zero-matmul clears has_written so
    overlapping chunk windows accumulate correctly).
  - emission is software-pipelined (filters of batch b emitted alongside
    matmul work of batch b-1) to keep the in-order engine queues busy.

"dense" (fallback, any input): same two-stage contraction without sorting
or windows, x/y symmetric on the full grid.
"""

import sys

import numpy as np

_B, _N, _C, _PS = 32, 64, 3, 16
_IMG, _PAD, _GRID = 256, 8, 272
_NCORES = 8
_BLOC = _B // _NCORES      # batches per core
_NCHUNK = 8                # stroke chunks
_CPB = _N // _NCHUNK       # strokes per chunk (8)
_CSCALE = 16.0             # coordinate downscale so d^2 fits fp16
_EXP_SCALE = -12.5 * _CSCALE * _CSCALE  # -1/(2*sigma^2) * CSCALE^2
_EPS = 1e-7

# static per-chunk x-windows (strokes sorted by gx; chunk j covers gx ranks
# 8j..8j+7). Window j = [woff_j, woff_j+128) in w; grid window adds the pad.
_WOFF = [min(max(32 * j - 48, 0), 128) for j in range(8)]
_WW = 128          # w-window width
_GW = _WW + 2 * _PAD  # grid window width for the x filters (144)

_cache = {}


def _fits_windows(brushes: np.ndarray) -> bool:
    """Check the static x-window scheme covers every stroke's support."""
    gx = np.sort(np.asarray(brushes, np.float32)[:, :, 0] * _IMG, axis=1)
    for j in range(8):
        lo = gx[:, 8 * j] * 1.0
        hi = gx[:, 8 * j + 7]
        if (np.maximum(lo - 10.0, 0.0) < _WOFF[j] - 1e-6).any():
            return False
        if (np.minimum(hi + 10.0, float(_IMG)) > _WOFF[j] + _WW + 1e-6).any():
            return False
    return True


def _build_nc(reps: int = 1, parts: str = "full", tiny_out: bool = False):
    # parts: "full" | "nos2" (skip stage-2/output) | "exponly" (filters only)
    # tiny_out: benchmark mode — real output goes to Internal DRAM so the
    # relay doesn't stream 25MB back per call; only a tiny tensor is External.
    if ("nc", reps, parts, tiny_out) in _cache:
        return _cache[("nc", reps, parts, tiny_out)]
    sys.path.insert(0, "/opt/trn_rl_repo")
    import concourse.tile as tile
    from concourse import bacc, mybir
    from contextlib import ExitStack

    fp32 = mybir.dt.float32
    fp16 = mybir.dt.float16
    AF = mybir.ActivationFunctionType
    OP = mybir.AluOpType

    nc = bacc.Bacc(
        "TRN2", target_bir_lowering=False, debug=False, enable_asserts=False
    )

    bigp = nc.dram_tensor(
        "bigp",
        [_BLOC, _C, 128, _NCHUNK * 128],
        fp16,
        kind="Internal" if tiny_out else "ExternalInput",
    ).ap()
    grepl = nc.dram_tensor(
        "grepl", [_BLOC, 128, 16], fp32, kind="ExternalInput"
    ).ap()
    cbc = nc.dram_tensor("cbc", [128, _GRID], fp16, kind="ExternalInput").ap()
    noffs = nc.dram_tensor("noffs", [128, 1], fp32, kind="ExternalInput").ap()
    outp = nc.dram_tensor(
        "outp",
        [_BLOC, _C, _IMG, _IMG],
        fp32,
        kind="Internal" if tiny_out else "ExternalOutput",
    ).ap()
    tiny = (
        nc.dram_tensor("tiny", [128, 4], fp32, kind="ExternalOutput").ap()
        if tiny_out
        else None
    )

    with tile.TileContext(nc) as tc, ExitStack() as ctx:
        cpool = ctx.enter_context(tc.tile_pool(name="const", bufs=1))
        gpool = ctx.enter_context(tc.tile_pool(name="g", bufs=4))
        mupool = ctx.enter_context(tc.tile_pool(name="mu", bufs=4))
        dpool = ctx.enter_context(tc.tile_pool(name="d", bufs=6))
        d2pool = ctx.enter_context(tc.tile_pool(name="d2", bufs=6))
        fpool = ctx.enter_context(tc.tile_pool(name="fraw", bufs=36))
        spool = ctx.enter_context(tc.tile_pool(name="sums", bufs=4))
        fnpool = ctx.enter_context(tc.tile_pool(name="fnorm", bufs=48))
        bppool = ctx.enter_context(tc.tile_pool(name="bp", bufs=3))
        uspool = ctx.enter_context(tc.tile_pool(name="us", bufs=3))
        obpool = ctx.enter_context(tc.tile_pool(name="ob", bufs=4))
        pupool = ctx.enter_context(tc.tile_pool(name="pu", bufs=2, space="PSUM"))
        popool = ctx.enter_context(tc.tile_pool(name="po", bufs=2, space="PSUM"))

        cb_t = cpool.tile([128, _GRID], fp16)
        nc.sync.dma_start(cb_t[:], cbc)
        no_t = cpool.tile([128, 1], fp32)
        nc.sync.dma_start(no_t[:], noffs)

        for _rep in range(reps):
          for b in range(_BLOC):
            g_t = gpool.tile([128, 16], fp32)
            nc.sync.dma_start(g_t[:], grepl[b])
            # negmu[:, ca] = (g * -256 - offs) / CSCALE  (per-partition AP)
            negmu = mupool.tile([128, 16], fp32)
            nc.vector.tensor_scalar(
                negmu[:], g_t[:], -256.0 / _CSCALE, no_t[:], OP.mult, OP.add
            )

            # y-axis filters first (ca 8..15) so stage 1 can start early;
            # per-axis sums tiles so sinv_y does not wait on x exps.
            f_raw = [None] * 16
            fn = [None] * 16
            sums_t = {}
            sinv_t = {}
            for axis in (1, 0):  # y first, then x
                sums = spool.tile([128, 8], fp32, tag=f"sums{axis}")
                sums_t[axis] = sums
                if parts == "exp_noaccum":
                    nc.vector.memset(sums[:], 1.0)
                for k in range(8):
                    ca = axis * 8 + k
                    d = dpool.tile([128, _GRID], fp16)
                    nc.vector.tensor_scalar(
                        d[:], cb_t[:], negmu[:, ca : ca + 1], None, OP.add
                    )
                    d2 = d2pool.tile([128, _GRID], fp16)
                    if parts == "exp_dvesq":
                        nc.vector.tensor_tensor(d2[:], d[:], d[:], OP.mult)
                    else:
                        nc.gpsimd.tensor_tensor(d2[:], d[:], d[:], OP.mult)
                    f = fpool.tile([128, _GRID], fp16)
                    nc.scalar.activation(
                        f[:],
                        d2[:],
                        AF.Exp,
                        scale=_EXP_SCALE,
                        accum_out=(None if parts == "exp_noaccum"
                                   else sums[:, k : k + 1]),
                    )
                    f_raw[ca] = f
                s2 = spool.tile([128, 8], fp32, tag=f"s2{axis}")
                nc.vector.tensor_scalar_add(s2[:], sums[:], _EPS)
                sinv = spool.tile([128, 8], fp32, tag=f"sinv{axis}")
                nc.vector.reciprocal(sinv[:], s2[:])
                sinv_t[axis] = sinv
                for k in range(8):
                    ca = axis * 8 + k
                    t = fnpool.tile([128, _IMG], fp16)
                    src = f_raw[ca][:, _PAD : _PAD + _IMG]
                    if axis == 0:  # x-axis filters (consumed by stage 2)
                        nc.vector.tensor_scalar(
                            t[:], src, sinv[:, k : k + 1], None, OP.mult
                        )
                    else:  # y-axis: fold in the 1/N stroke mean
                        nc.vector.tensor_scalar(
                            t[:],
                            src,
                            sinv[:, k : k + 1],
                            1.0 / _N,
                            OP.mult,
                            OP.mult,
                        )
                    fn[ca] = t

            if parts.startswith("exp"):
                # consume fn so nothing is dead: tiny copy to a scratch tile
                scr = obpool.tile([128, 1], fp32, tag="scr")
                nc.vector.tensor_copy(scr[:], fn[0][:, 0:1])
                nc.sync.dma_start(outp[b, 0, 0:128, 0:1], scr[:])
                if tiny_out and b == _BLOC - 1:
                    nc.sync.dma_start(tiny[:, 0:1], scr[:])
                continue
            for c in range(_C):
                bp = bppool.tile([128, _NCHUNK * 128], fp16)
                nc.sync.dma_start(bp[:], bigp[b, c])

                us_g = []
                for g in range(2):
                    pu = pupool.tile([128, 4 * _IMG], fp32)
                    for kk in range(4):
                        k = 4 * g + kk
                        # U^T[(i,q), h] = sum_(i,p) BigP_k[(i,p),(i,q)] FyN_k[(i,p), h]
                        nc.tensor.matmul(
                            pu[:, kk * _IMG : (kk + 1) * _IMG],
                            bp[:, k * 128 : (k + 1) * 128],
                            fn[8 + k][:],
                            start=True,
                            stop=True,
                        )
                    us = uspool.tile([128, 4 * _IMG], fp16)
                    if (c * 2 + g) % 2 == 0:
                        nc.scalar.copy(us[:], pu[:])
                    else:
                        nc.vector.tensor_copy(us[:], pu[:])
                    us_g.append(us)
                if parts == "nos2":
                    scr = obpool.tile([128, 1], fp32, tag="scr")
                    nc.vector.tensor_copy(scr[:], us_g[0][:, 0:1])
                    nc.sync.dma_start(outp[b, c, 0:128, 0:1], scr[:])
                    if tiny_out and b == _BLOC - 1 and c == _C - 1:
                        nc.sync.dma_start(tiny[:, 0:1], scr[:])
                    continue
                # one PSUM bank (tile) per output h-half; accumulation groups
                # must not interleave within a bank (bank-level start/stop)
                for hh in range(2):
                    po = popool.tile([128, _IMG], fp32, tag=f"po{hh}")
                    for g in range(2):
                        for kk in range(4):
                            k = 4 * g + kk
                            # out[h, w] += U^T.T @ FxN_k
                            nc.tensor.matmul(
                                po[:],
                                us_g[g][
                                    :,
                                    kk * _IMG + hh * 128 : kk * _IMG + hh * 128 + 128,
                                ],
                                fn[k][:],
                                start=(k == 0),
                                stop=(k == 7),
                            )
                    ob = obpool.tile([128, _IMG], fp32)
                    nc.vector.tensor_copy(ob[:], po[:])
                    nc.sync.dma_start(
                        outp[b, c, hh * 128 : (hh + 1) * 128, :], ob[:]
                    )
                    if tiny_out and b == _BLOC - 1 and c == _C - 1 and hh == 1:
                        nc.sync.dma_start(tiny[:, 0:4], ob[:, 0:4])

    nc.compile()
    _cache[("nc", reps, parts, tiny_out)] = nc
    return nc


def _host_pack(brushes: np.ndarray, patches: np.ndarray):
    """Shard + repack inputs for the 8 cores (layout only, no math)."""
    brushes = np.asarray(brushes, np.float32)
    patches = np.asarray(patches, np.float32)

    # coordinate grid constants (shared by all cores), pre-divided by CSCALE
    cbc = np.broadcast_to(
        ((np.arange(_GRID, dtype=np.float32) - _PAD) / _CSCALE)[None, :],
        (128, _GRID),
    ).astype(np.float16)
    noffs = (
        (7.5 - (np.arange(128, dtype=np.float32) % 16)) / _CSCALE
    ).reshape(128, 1)

    in_maps = []
    for cid in range(_NCORES):
        bs = brushes[cid * _BLOC : (cid + 1) * _BLOC]  # [4, 64, 2]
        ps = patches[cid * _BLOC : (cid + 1) * _BLOC]  # [4, 64, 3, 16, 16]

        # block-diagonal patch operand: A[b, c, 16i+p, k*128 + 16i+q]
        A = np.zeros((_BLOC, _C, 128, _NCHUNK, 128), np.float16)
        P6 = ps.reshape(_BLOC, _NCHUNK, _CPB, _C, _PS, _PS)  # [b,k,i,c,p,q]
        for i in range(_CPB):
            A[:, :, 16 * i : 16 * (i + 1), :, 16 * i : 16 * (i + 1)] = P6[
                :, :, i
            ].transpose(0, 2, 3, 1, 4)
        A = A.reshape(_BLOC, _C, 128, _NCHUNK * 128)

        # per-partition stroke centers: grepl[b, 16i+r, axis*8+k] = brushes[b, 8k+i, axis]
        G = bs.reshape(_BLOC, _NCHUNK, _CPB, 2)  # [b, k, i, axis]
        G = G.transpose(0, 2, 3, 1).reshape(_BLOC, _CPB, 16)  # [b, i, (axis,k)]
        grepl = np.repeat(G, 16, axis=1).astype(np.float32)  # [b, 128, 16]

        in_maps.append(
            {
                "bigp": A,
                "grepl": grepl,
                "cbc": cbc,
                "noffs": noffs,
            }
        )
    return in_maps


def _run(brushes, patches, trace=False, variant="auto"):
    if variant == "auto":
        variant = "win2" if _fits_windows(brushes) else "dense"
    sys.path.insert(0, "/opt/trn_rl_repo")
    from concourse import bass_utils

    if variant == "win2":
        nc = _build_win2_nc()
        in_maps = _host_pack_win2(brushes, patches)
    elif variant == "win":
        nc = _build_win_nc()
        in_maps = _host_pack_win(brushes, patches)
    else:
        nc = _build_nc()
        in_maps = _host_pack(brushes, patches)
    res = bass_utils.run_bass_kernel_spmd(
        nc, in_maps, core_ids=list(range(_NCORES)), trace=trace
    )
    outs = [res.results[cid]["outp"] for cid in range(_NCORES)]
    full = np.concatenate(outs, axis=0).astype(np.float32)
    if variant == "win2":
        # device stores [BLOC, C, 128, 2, 256] = (h%128, h//128, w)
        full = (
            full.reshape(_B, _C, 128, 2, _IMG)
            .transpose(0, 1, 3, 2, 4)
            .reshape(_B, _C, _IMG, _IMG)
        )
    return np.ascontiguousarray(full), res


def kernel(brushes: np.ndarray, patches: np.ndarray) -> np.ndarray:
    out, _ = _run(brushes, patches, trace=False)
    return out


def _build_win2_nc(reps: int = 1, tiny_out: bool = False):
    """win2: host folds BOTH filter normalizations (and the 1/N mean) into
    the packed patch operand, so the device pipeline is just:
      d = cb - mu (DVE), d^2 (DVE), one merged Exp per axis (ACT),
      stage-1 matmuls, us copy (DVE/GPSIMD round-robin),
      stage-2 matmuls (chunks 0 & 6 exactly tile [0,256) -> start=True,
      no zero-matmul), output DMA straight from PSUM.
    DMA count minimized (one grepl load, per-batch bigp loads, per-(b,c)
    output stores in a [128, 2, 256] device layout the host re-interleaves).
    """
    key = ("win2", reps, tiny_out)
    if key in _cache:
        return _cache[key]
    sys.path.insert(0, "/opt/trn_rl_repo")
    import concourse.tile as tile
    from concourse import bacc, mybir
    from contextlib import ExitStack

    fp32 = mybir.dt.float32
    fp16 = mybir.dt.float16
    AF = mybir.ActivationFunctionType
    OP = mybir.AluOpType

    nc = bacc.Bacc(
        "TRN2", target_bir_lowering=False, debug=False, enable_asserts=False
    )

    bigp = nc.dram_tensor(
        "bigp",
        [_BLOC, 128, _C * _NCHUNK * 128],
        fp16,
        kind="Internal" if tiny_out else "ExternalInput",
    ).ap()
    grepl = nc.dram_tensor(
        "grepl", [128, _BLOC * 16], fp32, kind="ExternalInput"
    ).ap()
    cbc = nc.dram_tensor("cbc", [128, _GRID], fp16, kind="ExternalInput").ap()
    noffs = nc.dram_tensor("noffs", [128, 1], fp32, kind="ExternalInput").ap()
    outp = nc.dram_tensor(
        "outp",
        [_BLOC, _C, 128, 2 * _IMG],
        fp32,
        kind="Internal" if tiny_out else "ExternalOutput",
    ).ap()
    tiny = (
        nc.dram_tensor("tiny", [128, 4], fp32, kind="ExternalOutput").ap()
        if tiny_out
        else None
    )

    with tile.TileContext(nc) as tc, ExitStack() as ctx:
        cpool = ctx.enter_context(tc.tile_pool(name="const", bufs=1))
        mupool = ctx.enter_context(tc.tile_pool(name="mu", bufs=2))
        dxpool = ctx.enter_context(tc.tile_pool(name="dx", bufs=2))
        d2xpool = ctx.enter_context(tc.tile_pool(name="d2x", bufs=2))
        fxpool = ctx.enter_context(tc.tile_pool(name="fx", bufs=2))
        dypool = ctx.enter_context(tc.tile_pool(name="dy", bufs=2))
        d2ypool = ctx.enter_context(tc.tile_pool(name="d2y", bufs=2))
        fypool = ctx.enter_context(tc.tile_pool(name="fy", bufs=2))
        bppool = ctx.enter_context(tc.tile_pool(name="bp", bufs=2))
        uspool = ctx.enter_context(tc.tile_pool(name="us", bufs=3))
        pupool = ctx.enter_context(tc.tile_pool(name="pu", bufs=2, space="PSUM"))
        popool = ctx.enter_context(tc.tile_pool(name="po", bufs=2, space="PSUM"))
        obpool = ctx.enter_context(tc.tile_pool(name="ob", bufs=3))

        cb_t = cpool.tile([128, _GRID], fp16)
        nc.sync.dma_start(cb_t[:], cbc)
        no_t = cpool.tile([128, 1], fp32)
        nc.sync.dma_start(no_t[:], noffs)
        zrow = cpool.tile([1, 512], fp16)
        nc.vector.memset(zrow[:], 0.0)

        for _rep in range(reps):
          g_all = mupool.tile([128, _BLOC * 16], fp32, tag="gall")
          nc.sync.dma_start(g_all[:], grepl)
          negmu = mupool.tile([128, _BLOC * 16], fp32, tag="negmu")
          nc.vector.tensor_scalar(
              negmu[:], g_all[:], -256.0 / _CSCALE, no_t[:], OP.mult, OP.add
          )

          pb = {}

          def emit_A(b):
            mu0 = b * 16
            dx = dxpool.tile([128, _NCHUNK * 128], fp16)
            for k in range(8):
                nc.vector.tensor_scalar(
                    dx[:, k * 128 : (k + 1) * 128],
                    cb_t[:, _WOFF[k] + _PAD : _WOFF[k] + _PAD + 128],
                    negmu[:, mu0 + k : mu0 + k + 1],
                    None,
                    OP.add,
                )
            d2x = d2xpool.tile([128, _NCHUNK * 128], fp16)
            nc.vector.tensor_tensor(d2x[:], dx[:], dx[:], OP.mult)
            fx = fxpool.tile([128, _NCHUNK * 128], fp16)
            nc.scalar.activation(fx[:], d2x[:], AF.Exp, scale=_EXP_SCALE)

            dy = dypool.tile([128, _NCHUNK * _IMG], fp16)
            for k in range(8):
                nc.vector.tensor_scalar(
                    dy[:, k * _IMG : (k + 1) * _IMG],
                    cb_t[:, _PAD : _PAD + _IMG],
                    negmu[:, mu0 + 8 + k : mu0 + 9 + k],
                    None,
                    OP.add,
                )
            d2y = d2ypool.tile([128, _NCHUNK * _IMG], fp16)
            nc.gpsimd.tensor_tensor(d2y[:], dy[:], dy[:], OP.mult)
            fy = fypool.tile([128, _NCHUNK * _IMG], fp16)
            nc.scalar.activation(fy[:], d2y[:], AF.Exp, scale=_EXP_SCALE)

            bp = bppool.tile([128, _C * _NCHUNK * 128], fp16)
            nc.sync.dma_start(bp[:], bigp[b])
            pb[b] = (fx, fy, bp)

          def emit_s1(b, c):
            fx, fy, bp = pb[b]
            pu = pupool.tile([128, _NCHUNK * 128], fp32)
            for k in range(8):
                nc.tensor.matmul(
                    pu[:, k * 128 : (k + 1) * 128],
                    bp[:, (c * 8 + k) * 128 : (c * 8 + k + 1) * 128],
                    fx[:, k * 128 : (k + 1) * 128],
                    start=True,
                    stop=True,
                )
            us = uspool.tile([128, _NCHUNK * 128], fp16)
            if c == 1:
                nc.vector.tensor_copy(us[:], pu[:])
            else:
                nc.scalar.copy(us[:], pu[:])
            return us

          def emit_s2(b, c, us):
            _, fy, _ = pb[b]
            po = popool.tile([128, 2 * _IMG], fp32)
            # zero + clear has_written via a K=1 zero matmul (proven on HW)
            nc.tensor.matmul(
                po[:],
                zrow[0:1, 0:128],
                zrow[0:1, 0:512],
                start=True,
                stop=False,
                skip_group_check=True,
            )
            for hh in range(2):
                for k in range(8):
                    nc.tensor.matmul(
                        po[:, hh * _IMG + _WOFF[k] : hh * _IMG + _WOFF[k] + _WW],
                        fy[:, k * _IMG + hh * 128 : k * _IMG + hh * 128 + 128],
                        us[:, k * 128 : (k + 1) * 128],
                        start=False,
                        stop=(hh == 1 and k == 7),
                        skip_group_check=True,
                    )
            ob = obpool.tile([128, 2 * _IMG], fp32)
            if c == 0:
                nc.scalar.copy(ob[:, 0:384], po[:, 0:384])
                nc.vector.tensor_copy(ob[:, 384:512], po[:, 384:512])
            else:
                nc.vector.tensor_copy(ob[:], po[:])
            nc.sync.dma_start(outp[b, c], ob[:])
            if tiny_out and b == _BLOC - 1 and c == _C - 1:
                nc.sync.dma_start(tiny[:], ob[:, 0:4])

          # software pipeline: filters(b+1) alongside matmuls(b); within a
          # batch, stage-1(c+1) is emitted before stage-2(c) so the PE queue
          # never waits on a PSUM->SBUF copy.
          steps = []
          for b in range(_BLOC):
              steps.append(("A", b))
              for c in range(_C):
                  steps.append(("M", b, c))
          us_q = []
          emit_A(0)
          for step in steps[1:] + [None]:
              if step is not None and step[0] == "A":
                  emit_A(step[1])
              else:
                  if step is not None:
                      _, b, c = step
                      us_q.append((b, c, emit_s1(b, c)))
                  if len(us_q) > (1 if step is not None else 0):
                      b0, c0, us0 = us_q.pop(0)
                      emit_s2(b0, c0, us0)
          while us_q:
              b0, c0, us0 = us_q.pop(0)
              emit_s2(b0, c0, us0)

    nc.compile()
    _cache[key] = nc
    return nc


def _host_pack_win2(brushes: np.ndarray, patches: np.ndarray):
    """Sort strokes by gx; fold both Gaussian-filter normalizations and the
    1/N stroke mean into the block-diagonal patch operand (host math is
    numpy-vectorized and exact-in-fp32, matching the reference)."""
    brushes = np.asarray(brushes, np.float32)
    patches = np.asarray(patches, np.float32)
    order = np.argsort(brushes[:, :, 0], axis=1)
    bidx = np.arange(_B)[:, None]
    brushes_s = brushes[bidx, order]              # [B, N, 2]
    patches_s = patches[bidx, order]              # [B, N, C, 16, 16]

    # normalization sums, exactly as the reference computes them (fp32)
    coords = np.arange(_GRID, dtype=np.float32) - _PAD          # [272]
    offs = np.arange(1, _PS + 1, dtype=np.float32) - _PS / 2 - 0.5  # [16]
    g = brushes_s * float(_IMG)                                  # [B, N, 2]
    mu = g[:, :, :, None] + offs[None, None, None, :]            # [B,N,2,16]
    F = np.exp(
        -((coords[None, None, None, None, :] - mu[..., None]) ** 2)
        / (2 * 0.2 ** 2)
    )                                                            # [B,N,2,16,272]
    sums = F.sum(axis=4) + _EPS                                  # [B,N,2,16]
    sinvx = 1.0 / sums[:, :, 0, :]                               # [B,N,16] per q
    sinvy = 1.0 / sums[:, :, 1, :] / _N                          # [B,N,16] per p

    cbc = np.broadcast_to(
        ((np.arange(_GRID, dtype=np.float32) - _PAD) / _CSCALE)[None, :],
        (128, _GRID),
    ).astype(np.float16)
    noffs = (
        (7.5 - (np.arange(128, dtype=np.float32) % 16)) / _CSCALE
    ).reshape(128, 1)

    in_maps = []
    for cid in range(_NCORES):
        sl = slice(cid * _BLOC, (cid + 1) * _BLOC)
        bs = brushes_s[sl]
        ps = patches_s[sl]
        sx = sinvx[sl].reshape(_BLOC, _NCHUNK, _CPB, _PS)   # [b,k,i,q]
        sy = sinvy[sl].reshape(_BLOC, _NCHUNK, _CPB, _PS)   # [b,k,i,p]

        P6 = ps.reshape(_BLOC, _NCHUNK, _CPB, _C, _PS, _PS)  # [b,k,i,c,p,q]
        P6 = P6 * sx[:, :, :, None, None, :] * sy[:, :, :, None, :, None]
        # A[b, 16i+q, c, k, 16i+p] block-diagonal
        A = np.zeros((_BLOC, 128, _C, _NCHUNK, 128), np.float16)
        for i in range(_CPB):
            A[:, 16 * i : 16 * (i + 1), :, :, 16 * i : 16 * (i + 1)] = P6[
                :, :, i
            ].transpose(0, 4, 2, 1, 3)  # [b, q, c, k, p]
        A = A.reshape(_BLOC, 128, _C * _NCHUNK * 128)

        G = bs.reshape(_BLOC, _NCHUNK, _CPB, 2)
        G = G.transpose(0, 2, 3, 1).reshape(_BLOC, _CPB, 16)  # [b, i, (axis,k)]
        grepl = np.repeat(G, 16, axis=1)                      # [b, 128, 16]
        grepl = grepl.transpose(1, 0, 2).reshape(128, _BLOC * 16)
        grepl = np.ascontiguousarray(grepl, np.float32)

        in_maps.append(
            {"bigp": A, "grepl": grepl, "cbc": cbc, "noffs": noffs}
        )
    return in_maps


def _build_win_nc(reps: int = 1, tiny_out: bool = False):
    """Windowed variant: strokes sorted by gx on host; stage 1 contracts q
    with transposed patch blocks and a static 128-wide w-window per chunk;
    stage 2 contracts (n,p) and scatters each chunk's w-window into the
    output PSUM (pre-zeroed by a K=1 zero-matmul so has_written semantics
    make overlapping windows accumulate)."""
    key = ("win", reps, tiny_out)
    if key in _cache:
        return _cache[key]
    sys.path.insert(0, "/opt/trn_rl_repo")
    import concourse.tile as tile
    from concourse import bacc, mybir
    from contextlib import ExitStack

    fp32 = mybir.dt.float32
    fp16 = mybir.dt.float16
    AF = mybir.ActivationFunctionType
    OP = mybir.AluOpType

    nc = bacc.Bacc(
        "TRN2", target_bir_lowering=False, debug=False, enable_asserts=False
    )

    bigp = nc.dram_tensor(
        "bigp",
        [_BLOC, _C, 128, _NCHUNK * 128],
        fp16,
        kind="Internal" if tiny_out else "ExternalInput",
    ).ap()
    grepl = nc.dram_tensor(
        "grepl", [_BLOC, 128, 16], fp32, kind="ExternalInput"
    ).ap()
    cbc = nc.dram_tensor("cbc", [128, _GRID], fp16, kind="ExternalInput").ap()
    noffs = nc.dram_tensor("noffs", [128, 1], fp32, kind="ExternalInput").ap()
    outp = nc.dram_tensor(
        "outp",
        [_BLOC, _C, _IMG, _IMG],
        fp32,
        kind="Internal" if tiny_out else "ExternalOutput",
    ).ap()
    tiny = (
        nc.dram_tensor("tiny", [128, 4], fp32, kind="ExternalOutput").ap()
        if tiny_out
        else None
    )

    with tile.TileContext(nc) as tc, ExitStack() as ctx:
        cpool = ctx.enter_context(tc.tile_pool(name="const", bufs=1))
        gpool = ctx.enter_context(tc.tile_pool(name="g", bufs=4))
        mupool = ctx.enter_context(tc.tile_pool(name="mu", bufs=4))
        dxpool = ctx.enter_context(tc.tile_pool(name="dx", bufs=6))
        dypool = ctx.enter_context(tc.tile_pool(name="dy", bufs=6))
        fxpool = ctx.enter_context(tc.tile_pool(name="fxraw", bufs=36))
        fypool = ctx.enter_context(tc.tile_pool(name="fyraw", bufs=36))
        spool = ctx.enter_context(tc.tile_pool(name="sums", bufs=6))
        fnxpool = ctx.enter_context(tc.tile_pool(name="fnx", bufs=24))
        fnypool = ctx.enter_context(tc.tile_pool(name="fny", bufs=24))
        bppool = ctx.enter_context(tc.tile_pool(name="bp", bufs=3))
        uspool = ctx.enter_context(tc.tile_pool(name="us", bufs=3))
        obpool = ctx.enter_context(tc.tile_pool(name="ob", bufs=4))
        pupool = ctx.enter_context(tc.tile_pool(name="pu", bufs=2, space="PSUM"))
        pxpool = ctx.enter_context(tc.tile_pool(name="px", bufs=2, space="PSUM"))
        popool = ctx.enter_context(tc.tile_pool(name="po", bufs=2, space="PSUM"))

        cb_t = cpool.tile([128, _GRID], fp16)
        nc.sync.dma_start(cb_t[:], cbc)
        no_t = cpool.tile([128, 1], fp32)
        nc.sync.dma_start(no_t[:], noffs)
        zrow = cpool.tile([1, 512], fp16)
        nc.vector.memset(zrow[:], 0.0)

        for _rep in range(reps):
          # software-pipelined emission: A(b) || B(b-1) so in-order engine
          # queues interleave raw-filter work with dependent per-batch work
          pb = {}

          def emit_A(b):
            g_t = gpool.tile([128, 16], fp32)
            nc.sync.dma_start(g_t[:], grepl[b])
            negmu = mupool.tile([128, 16], fp32)
            nc.vector.tensor_scalar(
                negmu[:], g_t[:], -256.0 / _CSCALE, no_t[:], OP.mult, OP.add
            )

            # x filters (windowed, consumed by stage 1): all-ACT chain
            # Square(cb + negmu) -> PSUM, Exp(PSUM) -> SBUF; no cross-engine
            # hops, exact fp32 squares.
            sums_x = spool.tile([128, 8], fp32, tag="sx")
            fx_raw = []
            for j in range(8):
                sq = pxpool.tile([128, _GW], fp32)
                nc.scalar.activation(
                    sq[:],
                    cb_t[:, _WOFF[j] : _WOFF[j] + _GW],
                    AF.Square,
                    bias=negmu[:, j : j + 1],
                    scale=1.0,
                )
                f = fxpool.tile([128, _GW], fp16)
                nc.scalar.activation(
                    f[:], sq[:], AF.Exp,
                    scale=_EXP_SCALE, accum_out=sums_x[:, j : j + 1],
                )
                fx_raw.append(f)

            # y filters (full grid, consumed by stage 2)
            sums_y = spool.tile([128, 8], fp32, tag="sy")
            fy_raw = []
            for k in range(8):
                d = dypool.tile([128, _GRID], fp16)
                nc.vector.tensor_scalar(
                    d[:], cb_t[:], negmu[:, 8 + k : 9 + k], None, OP.add
                )
                d2 = dypool.tile([128, _GRID], fp16, tag="dy2")
                nc.vector.tensor_tensor(d2[:], d[:], d[:], OP.mult)
                f = fypool.tile([128, _GRID], fp16)
                nc.scalar.activation(
                    f[:], d2[:], AF.Exp,
                    scale=_EXP_SCALE, accum_out=sums_y[:, k : k + 1],
                )
                fy_raw.append(f)
            pb[b] = (sums_x, fx_raw, sums_y, fy_raw)

          def emit_B(b):
            sums_x, fx_raw, sums_y, fy_raw = pb[b]
            s2x = spool.tile([128, 8], fp32, tag="s2x")
            nc.vector.tensor_scalar_add(s2x[:], sums_x[:], _EPS)
            sinv_x = spool.tile([128, 8], fp32, tag="six")
            nc.vector.reciprocal(sinv_x[:], s2x[:])
            fnx = []
            for j in range(8):
                t = fnxpool.tile([128, _WW], fp16)
                nc.vector.tensor_scalar(
                    t[:],
                    fx_raw[j][:, _PAD : _PAD + _WW],
                    sinv_x[:, j : j + 1],
                    None,
                    OP.mult,
                )
                fnx.append(t)
            s2y = spool.tile([128, 8], fp32, tag="s2y")
            nc.vector.tensor_scalar_add(s2y[:], sums_y[:], _EPS)
            sinv_y = spool.tile([128, 8], fp32, tag="siy")
            nc.vector.reciprocal(sinv_y[:], s2y[:])
            fny = []
            for k in range(8):
                t = fnypool.tile([128, _IMG], fp16)
                nc.vector.tensor_scalar(
                    t[:],
                    fy_raw[k][:, _PAD : _PAD + _IMG],
                    sinv_y[:, k : k + 1],
                    1.0 / _N,
                    OP.mult,
                    OP.mult,
                )
                fny.append(t)

            for c in range(_C):
                bp = bppool.tile([128, _NCHUNK * 128], fp16)
                nc.sync.dma_start(bp[:], bigp[b, c])

                pu = pupool.tile([128, 8 * _WW], fp32)
                for k in range(8):
                    # U'[(i,p), wl] = sum_(i,q) BigP2_k[(i,q),(i,p)] FxN_k[(i,q), wl]
                    nc.tensor.matmul(
                        pu[:, k * _WW : (k + 1) * _WW],
                        bp[:, k * 128 : (k + 1) * 128],
                        fnx[k][:],
                        start=True,
                        stop=True,
                    )
                us = uspool.tile([128, 8 * _WW], fp16)
                nc.vector.tensor_copy(us[:], pu[:])
                us_g = [us]

                # both h-halves share one PSUM bank: po[:, hh*256 + w]
                po = popool.tile([128, 2 * _IMG], fp32)
                # zero + clear has_written via a K=1 zero matmul
                nc.tensor.matmul(
                    po[:],
                    zrow[0:1, 0:128],
                    zrow[0:1, 0:512],
                    start=True,
                    stop=False,
                    skip_group_check=True,
                )
                for k in range(8):
                    for hh in range(2):
                        # out[h, woff_k + wl] += FyN_k^T @ U'_k
                        nc.tensor.matmul(
                            po[:, hh * _IMG + _WOFF[k] : hh * _IMG + _WOFF[k] + _WW],
                            fny[k][:, hh * 128 : hh * 128 + 128],
                            us_g[0][:, k * _WW : (k + 1) * _WW],
                            start=False,
                            stop=(k == 7 and hh == 1),
                            skip_group_check=True,
                        )
                ob = obpool.tile([128, 2 * _IMG], fp32)
                nc.vector.tensor_copy(ob[:], po[:])
                for hh in range(2):
                    nc.sync.dma_start(
                        outp[b, c, hh * 128 : (hh + 1) * 128, :],
                        ob[:, hh * _IMG : (hh + 1) * _IMG],
                    )
                if tiny_out and b == _BLOC - 1 and c == _C - 1:
                    nc.sync.dma_start(tiny[:, 0:4], ob[:, 0:4])

          for b in range(_BLOC):
            emit_A(b)
            if b >= 1:
                emit_B(b - 1)
          emit_B(_BLOC - 1)

    nc.compile()
    _cache[key] = nc
    return nc


def _host_pack_win(brushes: np.ndarray, patches: np.ndarray):
    """Like _host_pack but strokes sorted by gx per batch and patch blocks
    transposed (stage 1 contracts q)."""
    brushes = np.asarray(brushes, np.float32)
    patches = np.asarray(patches, np.float32)
    order = np.argsort(brushes[:, :, 0], axis=1)  # [B, N]
    bidx = np.arange(_B)[:, None]
    brushes_s = brushes[bidx, order]              # [B, N, 2]
    patches_s = patches[bidx, order]              # [B, N, C, 16, 16]

    cbc = np.broadcast_to(
        ((np.arange(_GRID, dtype=np.float32) - _PAD) / _CSCALE)[None, :],
        (128, _GRID),
    ).astype(np.float16)
    noffs = (
        (7.5 - (np.arange(128, dtype=np.float32) % 16)) / _CSCALE
    ).reshape(128, 1)

    in_maps = []
    for cid in range(_NCORES):
        bs = brushes_s[cid * _BLOC : (cid + 1) * _BLOC]
        ps = patches_s[cid * _BLOC : (cid + 1) * _BLOC]

        # transposed blocks: A2[b, c, 16i+q, k, 16i+p] = P[b, 8k+i, c, p, q]
        A = np.zeros((_BLOC, _C, 128, _NCHUNK, 128), np.float16)
        P6 = ps.reshape(_BLOC, _NCHUNK, _CPB, _C, _PS, _PS)  # [b,k,i,c,p,q]
        for i in range(_CPB):
            A[:, :, 16 * i : 16 * (i + 1), :, 16 * i : 16 * (i + 1)] = P6[
                :, :, i
            ].transpose(0, 2, 4, 1, 3)  # [b, c, q, k, p]
        A = A.reshape(_BLOC, _C, 128, _NCHUNK * 128)

        G = bs.reshape(_BLOC, _NCHUNK, _CPB, 2)
        G = G.transpose(0, 2, 3, 1).reshape(_BLOC, _CPB, 16)
        grepl = np.repeat(G, 16, axis=1).astype(np.float32)

        in_maps.append(
            {"bigp": A, "grepl": grepl, "cbc": cbc, "noffs": noffs}
        )
    return in_maps

